# revision 16
# baseline (speedup 1.0000x reference)
"""Trainium2 Bass kernel for nn_Correct_PrototypeManager (segment_reduce).

Reference computation:
    pred_lbl = argmax(preds, axis=1)                      # [B, H, W]
    feats_up = bilinear_resize(feats, H, W)               # [B, C, H, W]
    joint[b,k,h,w] = (masks==k) & (pred_lbl==k)
    counts[b,k] = sum_hw joint ; sums[b,k,c] = sum_hw feats_up * joint
    proto = mean_b( sums / (counts + eps) )               # [K, C]

Key algebraic transform: bilinear upsample is linear, so instead of
upsampling feats we DOWNSAMPLE the one-hot joint map with the adjoint
of the upsample (rows of U sum to 1, so counts are preserved exactly;
U's weights are quarters, so all downsample arithmetic is exact in
bf16 + f32 psum accumulation).

v2 layout strategy (all host prep is layout/cast only):
  - preds shipped pre-transposed to [hf, k, wf] so the DMA is fully
    contiguous and the argmax runs on natural views.
  - feats shipped bf16, pre-transposed to pixel-major [128, 32, 260]
    chunks with a ones-column at col 256 so the final contraction
    produces sums AND counts in one psum, with no on-device
    transposes of feats at all.
  - front-end (argmax eq / mask one-hot / joint) is split by class
    range across the DVE and GpSimd engines, which run in parallel.
  - final contraction is out[k, c]-oriented: 32 matmuls of 260 moving
    cols in 4 parallel psum chains.

Sharding: data-parallel over batch B=8, one image per NeuronCore; the
[K, C+1] per-image partial (sums | counts) is gathered on host,
divided and batch-meaned there (tiny).
"""

import numpy as np

B = 8
C = 256
K = 21
HC = WC = 64
HF = WF = 256
EPS = 1e-6
N_CORES = 8
PIX = HC * WC  # 4096
KW = K * WF    # 5376
K2 = K + 1     # 22 (pad class dim to even)
HK = HC * K2   # 1408
FTW = C + 4    # 260: feats cols + ones col (256) + pad
KSPL = 13      # eq/mul classes 0:13 on DVE, 13:21 on gpsimd

_PROGRAM_CACHE: dict = {}


def _upsample_matrix(n_in: int, n_out: int) -> np.ndarray:
    """U [n_out, n_in] with resize(x, 'bilinear', half-pixel) == U @ x."""
    U = np.zeros((n_out, n_in), dtype=np.float64)
    scale = n_in / n_out
    for i in range(n_out):
        src = (i + 0.5) * scale - 0.5
        f = int(np.floor(src))
        w = src - f
        lo = min(max(f, 0), n_in - 1)
        hi = min(max(f + 1, 0), n_in - 1)
        U[i, lo] += 1.0 - w
        U[i, hi] += w
    return U.astype(np.float32)


def _build_program(stage: int = 99):
    import concourse.bass as bass
    import concourse.bacc as bacc
    import concourse.tile as tile
    from concourse import mybir
    from contextlib import ExitStack

    f32 = mybir.dt.float32
    bf16 = mybir.dt.bfloat16

    nc = bacc.Bacc("TRN2", target_bir_lowering=False, debug=False,
                   num_devices=N_CORES)

    preds_d = nc.dram_tensor("preds", [2, 128, KW], f32,
                             kind="ExternalInput")
    mio_d = nc.dram_tensor("mio", [2, 128, WF + K], bf16,
                           kind="ExternalInput")
    ft_d = nc.dram_tensor("ft", [128, 32 * FTW], bf16, kind="ExternalInput")
    u_d = nc.dram_tensor("u", [2, 128, HC], bf16, kind="ExternalInput")
    ident_d = nc.dram_tensor("ident", [128, 128], bf16, kind="ExternalInput")
    out_d = nc.dram_tensor("out", [K, C + 1], f32, kind="ExternalOutput")
    if stage < 99:
        dbg_d = nc.dram_tensor("dbg", [128, KW], f32, kind="ExternalOutput")

    with tile.TileContext(nc) as tc, ExitStack() as ctx:
        const_pool = ctx.enter_context(tc.tile_pool(name="const", bufs=1))
        joint_pool = ctx.enter_context(tc.tile_pool(name="joint", bufs=1))
        res_pool = ctx.enter_context(tc.tile_pool(name="res", bufs=1))
        ps_pool = ctx.enter_context(
            tc.tile_pool(name="ps", bufs=2, space="PSUM"))
        psf_pool = ctx.enter_context(
            tc.tile_pool(name="psf", bufs=1, space="PSUM"))

        # --- constants / inputs ---
        # preds DMA first so the front-end can start ASAP
        preds_t = []
        with tc.tile_pool(name="preds", bufs=1) as pr_pool:
            for h in range(2):
                pt = pr_pool.tile([128, KW], f32, tag=f"preds{h}")
                nc.sync.dma_start(pt[:], preds_d.ap()[h, :, :])
                preds_t.append(pt)

            u_t = []
            for h in range(2):
                ut = const_pool.tile([128, HC], bf16, tag=f"u{h}")
                nc.sync.dma_start(ut[:], u_d.ap()[h, :, :])
                u_t.append(ut)
            ident_t = const_pool.tile([128, 128], bf16, tag="ident")
            nc.sync.dma_start(ident_t[:], ident_d.ap()[:, :])
            ft_big = const_pool.tile([128, 32 * FTW], bf16, tag="ftbig")
            nc.sync.dma_start(ft_big[:], ft_d.ap()[:, :])
            mio_t = []
            for h in range(2):
                mt = const_pool.tile([128, WF + K], bf16, tag=f"mio{h}")
                nc.sync.dma_start(mt[:], mio_d.ap()[h, :, :])
                mio_t.append(mt)

            # --- front-end: joint one-hot map, split DVE / gpsimd ---
            # gpsimd builds the mask one-hot (only needs mio, which lands
            # early); DVE computes the class max via a contiguous TT-max
            # tree; eq and the joint mul are split by class range.
            joint_t = []
            with tc.tile_pool(name="fe", bufs=2) as fe_pool:
                for h in range(2):
                    p3 = preds_t[h][:].rearrange("p (k w) -> p k w", k=K)
                    mask_t = mio_t[h]

                    oh_t = fe_pool.tile([128, KW], bf16, tag="oh")
                    o3 = oh_t[:].rearrange("p (k w) -> p k w", k=K)
                    nc.vector.tensor_tensor(
                        o3[:, :, :],
                        mask_t[:, :WF].unsqueeze(1).to_broadcast(
                            [128, K, WF]),
                        mask_t[:, WF:WF + K].unsqueeze(2)
                        .to_broadcast([128, K, WF]),
                        op=mybir.AluOpType.is_equal)

                    # TT-max tree over classes (contiguous views)
                    mx = fe_pool.tile([128, 11 * WF], f32, tag="mx")
                    m3 = mx[:].rearrange("p (k w) -> p k w", k=11)
                    nc.vector.tensor_tensor(
                        m3[:, 0:10, :], p3[:, 0:10, :], p3[:, 11:21, :],
                        op=mybir.AluOpType.max)
                    nc.vector.tensor_copy(m3[:, 10, :], p3[:, 10, :])
                    nc.vector.tensor_tensor(
                        m3[:, 0:5, :], m3[:, 0:5, :], m3[:, 6:11, :],
                        op=mybir.AluOpType.max)
                    nc.vector.tensor_tensor(
                        m3[:, 0:3, :], m3[:, 0:3, :], m3[:, 3:6, :],
                        op=mybir.AluOpType.max)
                    nc.vector.tensor_tensor(
                        m3[:, 0, :], m3[:, 0, :], m3[:, 1, :],
                        op=mybir.AluOpType.max)
                    nc.vector.tensor_tensor(
                        m3[:, 0, :], m3[:, 0, :], m3[:, 2, :],
                        op=mybir.AluOpType.max)
                    maxv = m3[:, 0, :]

                    eq_t = fe_pool.tile([128, KW], bf16, tag="eq")
                    jt = joint_pool.tile([128, KW], bf16, tag=f"joint{h}")
                    e3 = eq_t[:].rearrange("p (k w) -> p k w", k=K)
                    j3 = jt[:].rearrange("p (k w) -> p k w", k=K)
                    nc.vector.tensor_tensor(
                        e3[:, :, :], p3[:, :, :],
                        maxv.unsqueeze(1).to_broadcast([128, K, WF]),
                        op=mybir.AluOpType.is_equal)
                    nc.vector.tensor_tensor(
                        j3[:, :, :], e3[:, :, :],
                        o3[:, :, :], op=mybir.AluOpType.mult)
                    joint_t.append(jt)

            if stage <= 1:  # debug: dump joint (cast bf16->f32 via gpsimd)
                nc.gpsimd.dma_start(dbg_d.ap()[:, :], joint_t[0][:, :])

        # ----- stage 1: contract hf.  A[hc, (k, wf)] = Uh^T @ joint -----
        with tc.tile_pool(name="stage", bufs=1) as st_pool:
            a_t = st_pool.tile([64, KW], bf16, tag="a")
            for ci, fc in enumerate(range(0, KW if stage >= 2 else 0, 512)):
                w = min(512, KW - fc)
                ps = ps_pool.tile([64, 512], f32, tag="ps")
                nc.tensor.matmul(ps[:, :w], u_t[0][:, :],
                                 joint_t[0][:, fc:fc + w],
                                 start=True, stop=False)
                nc.tensor.matmul(ps[:, :w], u_t[1][:, :],
                                 joint_t[1][:, fc:fc + w],
                                 start=False, stop=True)
                nc.scalar.copy(a_t[:, fc:fc + w], ps[:, :w])
            if stage == 2:
                nc.gpsimd.dma_start(dbg_d.ap()[0:64, 0:KW], a_t[:, :])

            # ----- stage 1.5: transpose A per class -> AT[wf, (wh,(hc,k2))]
            at_big = st_pool.tile([128, 2 * HK], bf16, tag="at")
            if stage >= 3:
                # zero the k=21 pad column so stage 2 produces clean zeros
                nc.gpsimd.memset(
                    at_big[:].rearrange(
                        "p (w h k) -> p w h k", w=2, h=HC)[:, :, :, K], 0.0)
            for k in range(K if stage >= 3 else 0):
                ps = ps_pool.tile([128, 128], bf16, tag="pst")
                for wh in range(2):
                    nc.tensor.transpose(
                        ps[:, wh * 64:(wh + 1) * 64],
                        a_t[:, k * WF + wh * 128: k * WF + wh * 128 + 128],
                        ident_t[:64, :64])
                dst = at_big[:].rearrange(
                    "p (w h k) -> p w h k", w=2, h=HC)[:, :, :, k]
                src = ps[:].rearrange("p (w h) -> p w h", w=2)
                nc.scalar.copy(dst, src)

            # ----- stage 2: contract wf.  B[wc, (hc, k)] = Uw^T @ AT -----
            # B lives twice (partitions 0-63 and 64-127) so the final
            # matmuls can match the base partition of the FT slice.
            b_t = st_pool.tile([128, HK], bf16, tag="b")
            for fc in range(0, HK if stage >= 3 else 0, 512):
                w = min(512, HK - fc)
                ps = ps_pool.tile([64, 512], f32, tag="ps")
                nc.tensor.matmul(ps[:, :w], u_t[0][:, :],
                                 at_big[:, fc:fc + w],
                                 start=True, stop=False)
                nc.tensor.matmul(ps[:, :w], u_t[1][:, :],
                                 at_big[:, HK + fc:HK + fc + w],
                                 start=False, stop=True)
                nc.scalar.copy(b_t[0:64, fc:fc + w], ps[:, :w])
            if stage >= 3:
                # partitions 64-127 hold B shifted by one hc, so one
                # 128-pixel chunk (two hc rows) is a single matmul slice
                nc.sync.dma_start(b_t[64:128, 0:HK - K2], b_t[0:64, K2:HK])
            if stage == 3:
                nc.gpsimd.dma_start(dbg_d.ap()[:, 0:HK], b_t[:, :])

            # ----- final: out[k, c|cnt] = sum_pix B[pix,k] (x) FT[pix,:] --
            ftv = ft_big[:].rearrange("p (x n) -> p x n", x=32)
            psf_t = []
            for cchain in range(4 if stage >= 4 else 0):
                psf = psf_pool.tile([K2, FTW], f32, tag=f"fin{cchain}")
                for i in range(8):
                    ch = cchain * 8 + i
                    nc.tensor.matmul(
                        psf[:, :],
                        b_t[:, 2 * ch * K2: 2 * ch * K2 + K2],
                        ftv[:, ch, :],
                        start=(i == 0), stop=(i == 7))
                psf_t.append(psf)

            if stage >= 4:
                # TT may read only one PSUM operand: copy then chain adds
                s01 = res_pool.tile([K2, FTW], f32, tag="s01")
                nc.vector.tensor_copy(s01[:], psf_t[0][:])
                for i in (1, 2, 3):
                    nc.vector.tensor_tensor(s01[:], s01[:], psf_t[i][:],
                                            op=mybir.AluOpType.add)
                nc.sync.dma_start(out_d.ap()[:, :], s01[0:K, 0:C + 1])

    nc.compile()
    return nc


def _get_program(stage: int = 99):
    key = f"nc{stage}"
    if key not in _PROGRAM_CACHE:
        _PROGRAM_CACHE[key] = _build_program(stage)
    return _PROGRAM_CACHE[key]


def _host_inputs(feats, preds, masks):
    import ml_dtypes

    U = _upsample_matrix(HC, HF)  # [256, 64], exact in bf16
    u_pack = np.ascontiguousarray(
        U.reshape(2, 128, HC)).astype(ml_dtypes.bfloat16)
    ident = np.eye(128, dtype=np.float32).astype(ml_dtypes.bfloat16)

    feats = np.asarray(feats, dtype=np.float32)
    preds = np.asarray(preds, dtype=np.float32)
    masks_f = np.asarray(masks).astype(np.float32)
    iota_row = np.arange(K, dtype=np.float32)
    # [B, 2, 128, WF+K]: mask halves with the iota row appended
    mio = np.empty((B, 2, 128, WF + K), dtype=np.float32)
    mio[..., :WF] = masks_f.reshape(B, 2, 128, WF)
    mio[..., WF:] = iota_row
    mio_bf = mio.astype(ml_dtypes.bfloat16)

    # preds pre-transposed to [hf, k, wf], split into halves
    preds_hf = np.ascontiguousarray(
        preds.transpose(0, 2, 1, 3)).reshape(B, 2, 128, KW)

    # feats: [C, PIX] -> pixel-major chunks [128, 32, FTW] bf16 + ones col
    ft_host = np.zeros((B, 128, 32, FTW), dtype=np.float32)
    ftt = feats.reshape(B, C, PIX).transpose(0, 2, 1)   # [B, PIX, C]
    ft_host[..., :C] = ftt.reshape(B, 32, 128, C).transpose(0, 2, 1, 3)
    ft_host[..., C] = 1.0
    ft_bf = ft_host.reshape(B, 128, 32 * FTW).astype(ml_dtypes.bfloat16)

    in_maps = []
    for b in range(B):
        in_maps.append({
            "preds": np.ascontiguousarray(preds_hf[b]),
            "mio": np.ascontiguousarray(mio_bf[b]),
            "ft": np.ascontiguousarray(ft_bf[b]),
            "u": u_pack,
            "ident": ident,
        })
    return in_maps


def kernel(feats, preds, masks, _results_hook=None, _stage=99):
    from concourse.bass_utils import run_bass_kernel_spmd

    nc = _get_program(_stage)
    in_maps = _host_inputs(feats, preds, masks)
    res = run_bass_kernel_spmd(nc, in_maps, list(range(N_CORES)))
    if _results_hook is not None:
        _results_hook(res)

    protos = []
    for b in range(B):
        out = res.results[b]["out"]   # [K, C+1] f32
        sums = out[:, :C]             # [K, C]
        counts = out[:, C]            # [K]
        protos.append(sums / (counts + EPS)[:, None])   # [K, C]
    return np.mean(np.stack(protos), axis=0).astype(np.float32)


# revision 21
# speedup vs baseline: 1.0222x; 1.0222x over previous
"""Trainium2 Bass kernel for nn_Correct_PrototypeManager (segment_reduce).

Reference computation:
    pred_lbl = argmax(preds, axis=1)                      # [B, H, W]
    feats_up = bilinear_resize(feats, H, W)               # [B, C, H, W]
    joint[b,k,h,w] = (masks==k) & (pred_lbl==k)
    counts[b,k] = sum_hw joint ; sums[b,k,c] = sum_hw feats_up * joint
    proto = mean_b( sums / (counts + eps) )               # [K, C]

Key algebraic transform: bilinear upsample is linear, so instead of
upsampling feats we DOWNSAMPLE the one-hot joint map with the adjoint
of the upsample (rows of U sum to 1, so counts are preserved exactly;
U's weights are quarters, so all downsample arithmetic is exact in
bf16 + f32 psum accumulation).

v2 layout strategy (all host prep is layout/cast only):
  - preds shipped pre-transposed to [hf, k, wf] so the DMA is fully
    contiguous and the argmax runs on natural views.
  - feats shipped bf16, pre-transposed to pixel-major [128, 32, 260]
    chunks with a ones-column at col 256 so the final contraction
    produces sums AND counts in one psum, with no on-device
    transposes of feats at all.
  - front-end (argmax eq / mask one-hot / joint) is split by class
    range across the DVE and GpSimd engines, which run in parallel.
  - final contraction is out[k, c]-oriented: 32 matmuls of 260 moving
    cols in 4 parallel psum chains.

Sharding: data-parallel over batch B=8, one image per NeuronCore; the
[K, C+1] per-image partial (sums | counts) is gathered on host,
divided and batch-meaned there (tiny).
"""

import numpy as np

B = 8
C = 256
K = 21
HC = WC = 64
HF = WF = 256
EPS = 1e-6
N_CORES = 8
PIX = HC * WC  # 4096
KW = K * WF    # 5376
K2 = K + 1     # 22 (pad class dim to even)
HK = HC * K2   # 1408
FTW = C + 4    # 260: feats cols + ones col (256) + pad
KSPL = 13      # eq/mul classes 0:13 on DVE, 13:21 on gpsimd

_PROGRAM_CACHE: dict = {}


def _upsample_matrix(n_in: int, n_out: int) -> np.ndarray:
    """U [n_out, n_in] with resize(x, 'bilinear', half-pixel) == U @ x."""
    U = np.zeros((n_out, n_in), dtype=np.float64)
    scale = n_in / n_out
    for i in range(n_out):
        src = (i + 0.5) * scale - 0.5
        f = int(np.floor(src))
        w = src - f
        lo = min(max(f, 0), n_in - 1)
        hi = min(max(f + 1, 0), n_in - 1)
        U[i, lo] += 1.0 - w
        U[i, hi] += w
    return U.astype(np.float32)


def _build_program(stage: int = 99):
    import concourse.bass as bass
    import concourse.bacc as bacc
    import concourse.tile as tile
    from concourse import mybir
    from contextlib import ExitStack

    f32 = mybir.dt.float32
    bf16 = mybir.dt.bfloat16

    nc = bacc.Bacc("TRN2", target_bir_lowering=False, debug=False,
                   num_devices=N_CORES)

    preds_d = nc.dram_tensor("preds", [2, 128, KW], f32,
                             kind="ExternalInput")
    mio_d = nc.dram_tensor("mio", [2, 128, WF + K], bf16,
                           kind="ExternalInput")
    ft_d = nc.dram_tensor("ft", [128, 32 * FTW], bf16, kind="ExternalInput")
    u_d = nc.dram_tensor("u", [2, 128, HC], bf16, kind="ExternalInput")
    ident_d = nc.dram_tensor("ident", [128, 128], bf16, kind="ExternalInput")
    out_d = nc.dram_tensor("out", [K, C + 1], f32, kind="ExternalOutput")
    if stage < 99:
        dbg_d = nc.dram_tensor("dbg", [128, KW], f32, kind="ExternalOutput")

    with tile.TileContext(nc) as tc, ExitStack() as ctx:
        const_pool = ctx.enter_context(tc.tile_pool(name="const", bufs=1))
        joint_pool = ctx.enter_context(tc.tile_pool(name="joint", bufs=1))
        res_pool = ctx.enter_context(tc.tile_pool(name="res", bufs=1))
        ps_pool = ctx.enter_context(
            tc.tile_pool(name="ps", bufs=2, space="PSUM"))
        psf_pool = ctx.enter_context(
            tc.tile_pool(name="psf", bufs=1, space="PSUM"))

        # --- constants / inputs ---
        # DMA priority: small tiles first, then preds_h0 (the head of the
        # dependency chain), preds_h1, and ft last (needed only at the end)
        mio_t = []
        for h in range(2):
            mt = const_pool.tile([128, WF + K], bf16, tag=f"mio{h}")
            nc.sync.dma_start(mt[:], mio_d.ap()[h, :, :])
            mio_t.append(mt)
        u_t = []
        for h in range(2):
            ut = const_pool.tile([128, HC], bf16, tag=f"u{h}")
            nc.sync.dma_start(ut[:], u_d.ap()[h, :, :])
            u_t.append(ut)
        ident_t = const_pool.tile([128, 128], bf16, tag="ident")
        nc.sync.dma_start(ident_t[:], ident_d.ap()[:, :])

        preds_t = []
        with tc.tile_pool(name="preds", bufs=1) as pr_pool:
            for h in range(2):
                pt = pr_pool.tile([128, KW], f32, tag=f"preds{h}")
                nc.sync.dma_start(pt[:], preds_d.ap()[h, :, :])
                preds_t.append(pt)
            ft_big = const_pool.tile([128, 32 * FTW], bf16, tag="ftbig")
            nc.sync.dma_start(ft_big[:], ft_d.ap()[:, :])

            # iota replicated [128, (k w)] bf16 via gpsimd memsets (idle
            # engine, runs during DMA); mask replicated per half via ACT
            # broadcast copies (also in the DMA shadow). With both
            # operands materialized the one-hot is a plain bf16 TT at
            # 2 elem/lane/cycle instead of broadcast ops at 1.
            iota_rep = const_pool.tile([128, KW], bf16, tag="iotarep")
            i3 = iota_rep[:].rearrange("p (k w) -> p k w", k=K)
            for k in range(K):
                nc.gpsimd.memset(i3[:, k, :], float(k))

            # --- front-end: joint one-hot map (DVE), ACT preps mask_rep ---
            joint_t = []
            with tc.tile_pool(name="fe", bufs=1) as fe_pool:
                for h in range(2):
                    p3 = preds_t[h][:].rearrange("p (k w) -> p k w", k=K)
                    mask_t = mio_t[h]

                    # materialized mask replica (ACT, in the DMA shadow)
                    mrep = fe_pool.tile([128, KW], bf16, tag="mrep")
                    nc.scalar.copy(
                        mrep[:].rearrange("p (k w) -> p k w", k=K),
                        mask_t[:, :WF].unsqueeze(1).to_broadcast(
                            [128, K, WF]))
                    oh_t = fe_pool.tile([128, KW], bf16, tag="oh")
                    nc.vector.tensor_tensor(
                        oh_t[:], mrep[:], iota_rep[:],
                        op=mybir.AluOpType.is_equal)

                    # TT-max tree over classes (contiguous views)
                    mx = fe_pool.tile([128, 11 * WF], f32, tag="mx")
                    m3 = mx[:].rearrange("p (k w) -> p k w", k=11)
                    nc.vector.tensor_tensor(
                        m3[:, 0:10, :], p3[:, 0:10, :], p3[:, 11:21, :],
                        op=mybir.AluOpType.max)
                    nc.vector.tensor_copy(m3[:, 10, :], p3[:, 10, :])
                    nc.vector.tensor_tensor(
                        m3[:, 0:5, :], m3[:, 0:5, :], m3[:, 6:11, :],
                        op=mybir.AluOpType.max)
                    nc.vector.tensor_tensor(
                        m3[:, 0:3, :], m3[:, 0:3, :], m3[:, 3:6, :],
                        op=mybir.AluOpType.max)
                    nc.vector.tensor_tensor(
                        m3[:, 0, :], m3[:, 0, :], m3[:, 1, :],
                        op=mybir.AluOpType.max)
                    nc.vector.tensor_tensor(
                        m3[:, 0, :], m3[:, 0, :], m3[:, 2, :],
                        op=mybir.AluOpType.max)
                    maxv = m3[:, 0, :]

                    eq_t = fe_pool.tile([128, KW], bf16, tag="eq")
                    jt = joint_pool.tile([128, KW], bf16, tag=f"joint{h}")
                    e3 = eq_t[:].rearrange("p (k w) -> p k w", k=K)
                    nc.vector.tensor_tensor(
                        e3[:, :, :], p3[:, :, :],
                        maxv.unsqueeze(1).to_broadcast([128, K, WF]),
                        op=mybir.AluOpType.is_equal)
                    nc.vector.tensor_tensor(
                        jt[:], eq_t[:], oh_t[:], op=mybir.AluOpType.mult)
                    joint_t.append(jt)

            if stage <= 1:  # debug: dump joint (cast bf16->f32 via gpsimd)
                nc.gpsimd.dma_start(dbg_d.ap()[:, :], joint_t[0][:, :])

        # ----- stage 1: contract hf.  A[hc, (k, wf)] = Uh^T @ joint -----
        # split per half: h0's matmuls + ACT copies run while the DVE is
        # still on h1's front-end; h1's matmuls then add into a_t via DVE
        # (idle in the tail).
        with tc.tile_pool(name="stage", bufs=1) as st_pool:
            a0_t = st_pool.tile([64, KW], bf16, tag="a0")
            a_t = st_pool.tile([64, KW], bf16, tag="a")
            for ci, fc in enumerate(range(0, KW if stage >= 2 else 0, 512)):
                w = min(512, KW - fc)
                ps = ps_pool.tile([64, 512], f32, tag="ps")
                nc.tensor.matmul(ps[:, :w], u_t[0][:, :],
                                 joint_t[0][:, fc:fc + w],
                                 start=True, stop=True)
                nc.scalar.copy(a0_t[:, fc:fc + w], ps[:, :w])
            for ci, fc in enumerate(range(0, KW if stage >= 2 else 0, 512)):
                w = min(512, KW - fc)
                ps = ps_pool.tile([64, 512], f32, tag="ps")
                nc.tensor.matmul(ps[:, :w], u_t[1][:, :],
                                 joint_t[1][:, fc:fc + w],
                                 start=True, stop=True)
                nc.vector.tensor_tensor(a_t[:, fc:fc + w],
                                        a0_t[:, fc:fc + w], ps[:, :w],
                                        op=mybir.AluOpType.add)
            if stage == 2:
                nc.gpsimd.dma_start(dbg_d.ap()[0:64, 0:KW], a_t[:, :])

            # ----- stage 1.5: transpose A per class -> AT[wf, (wh,(hc,k2))]
            at_big = st_pool.tile([128, 2 * HK], bf16, tag="at")
            if stage >= 3:
                # zero the k=21 pad column so stage 2 produces clean zeros
                nc.gpsimd.memset(
                    at_big[:].rearrange(
                        "p (w h k) -> p w h k", w=2, h=HC)[:, :, :, K], 0.0)
            for k in range(K if stage >= 3 else 0):
                ps = ps_pool.tile([128, 128], bf16, tag="pst")
                for wh in range(2):
                    nc.tensor.transpose(
                        ps[:, wh * 64:(wh + 1) * 64],
                        a_t[:, k * WF + wh * 128: k * WF + wh * 128 + 128],
                        ident_t[:64, :64])
                dst = at_big[:].rearrange(
                    "p (w h k) -> p w h k", w=2, h=HC)[:, :, :, k]
                src = ps[:].rearrange("p (w h) -> p w h", w=2)
                if k % 3 == 2:
                    nc.scalar.copy(dst, src)
                else:
                    nc.vector.tensor_copy(dst, src)

            # ----- stage 2: contract wf.  B[wc, (hc, k)] = Uw^T @ AT -----
            # B lives twice (partitions 0-63 and 64-127) so the final
            # matmuls can match the base partition of the FT slice.
            # 4 chunks of 352 cols (16 hc) aligned with the 4 final psum
            # chains; the partition-shift dup runs per chunk on gpsimd so
            # chain c can start right after dup c.
            b_t = st_pool.tile([128, HK], bf16, tag="b")
            CW = HK // 4  # 352
            for fc in range(0, HK if stage >= 3 else 0, CW):
                ps = ps_pool.tile([64, 512], f32, tag="ps")
                nc.tensor.matmul(ps[:, :CW], u_t[0][:, :],
                                 at_big[:, fc:fc + CW],
                                 start=True, stop=False)
                nc.tensor.matmul(ps[:, :CW], u_t[1][:, :],
                                 at_big[:, HK + fc:HK + fc + CW],
                                 start=False, stop=True)
                nc.scalar.copy(b_t[0:64, fc:fc + CW], ps[:, :CW])
                # dup for this chunk: dst cols [fc-K2, fc+CW-K2) from
                # src cols [fc, fc+CW) shifted by one hc (K2 cols)
                d0 = max(fc - K2, 0)
                nc.gpsimd.dma_start(b_t[64:128, d0:fc + CW - K2],
                                    b_t[0:64, d0 + K2:fc + CW])
            if stage == 3:
                nc.gpsimd.dma_start(dbg_d.ap()[:, 0:HK], b_t[:, :])

            # ----- final: out[k, c|cnt] = sum_pix B[pix,k] (x) FT[pix,:] --
            # 4 chains emitted round-robin; chain c consumes b_t chunk c.
            ftv = ft_big[:].rearrange("p (x n) -> p x n", x=32)
            psf_t = []
            if stage >= 4:
                for c in range(4):
                    psf = psf_pool.tile([K2, FTW], f32, tag=f"fin{c}")
                    psf_t.append(psf)
                for i in range(8):
                    for c in range(4):
                        ch = c * 8 + i
                        nc.tensor.matmul(
                            psf_t[c][:, :],
                            b_t[:, 2 * ch * K2: 2 * ch * K2 + K2],
                            ftv[:, ch, :],
                            start=(i == 0), stop=(i == 7))

            if stage >= 4:
                # TT may read only one PSUM operand: copy then chain adds
                s01 = res_pool.tile([K2, FTW], f32, tag="s01")
                nc.vector.tensor_copy(s01[:], psf_t[0][:])
                for i in (1, 2, 3):
                    nc.vector.tensor_tensor(s01[:], s01[:], psf_t[i][:],
                                            op=mybir.AluOpType.add)
                nc.sync.dma_start(out_d.ap()[:, :], s01[0:K, 0:C + 1])

    nc.compile()
    return nc


def _get_program(stage: int = 99):
    key = f"nc{stage}"
    if key not in _PROGRAM_CACHE:
        _PROGRAM_CACHE[key] = _build_program(stage)
    return _PROGRAM_CACHE[key]


def _host_inputs(feats, preds, masks):
    import ml_dtypes

    U = _upsample_matrix(HC, HF)  # [256, 64], exact in bf16
    u_pack = np.ascontiguousarray(
        U.reshape(2, 128, HC)).astype(ml_dtypes.bfloat16)
    ident = np.eye(128, dtype=np.float32).astype(ml_dtypes.bfloat16)

    feats = np.asarray(feats, dtype=np.float32)
    preds = np.asarray(preds, dtype=np.float32)
    masks_f = np.asarray(masks).astype(np.float32)
    iota_row = np.arange(K, dtype=np.float32)
    # [B, 2, 128, WF+K]: mask halves with the iota row appended
    mio = np.empty((B, 2, 128, WF + K), dtype=np.float32)
    mio[..., :WF] = masks_f.reshape(B, 2, 128, WF)
    mio[..., WF:] = iota_row
    mio_bf = mio.astype(ml_dtypes.bfloat16)

    # preds pre-transposed to [hf, k, wf], split into halves
    preds_hf = np.ascontiguousarray(
        preds.transpose(0, 2, 1, 3)).reshape(B, 2, 128, KW)

    # feats: [C, PIX] -> pixel-major chunks [128, 32, FTW] bf16 + ones col
    ft_host = np.zeros((B, 128, 32, FTW), dtype=np.float32)
    ftt = feats.reshape(B, C, PIX).transpose(0, 2, 1)   # [B, PIX, C]
    ft_host[..., :C] = ftt.reshape(B, 32, 128, C).transpose(0, 2, 1, 3)
    ft_host[..., C] = 1.0
    ft_bf = ft_host.reshape(B, 128, 32 * FTW).astype(ml_dtypes.bfloat16)

    in_maps = []
    for b in range(B):
        in_maps.append({
            "preds": np.ascontiguousarray(preds_hf[b]),
            "mio": np.ascontiguousarray(mio_bf[b]),
            "ft": np.ascontiguousarray(ft_bf[b]),
            "u": u_pack,
            "ident": ident,
        })
    return in_maps


def kernel(feats, preds, masks, _results_hook=None, _stage=99):
    from concourse.bass_utils import run_bass_kernel_spmd

    nc = _get_program(_stage)
    in_maps = _host_inputs(feats, preds, masks)
    res = run_bass_kernel_spmd(nc, in_maps, list(range(N_CORES)))
    if _results_hook is not None:
        _results_hook(res)

    protos = []
    for b in range(B):
        out = res.results[b]["out"]   # [K, C+1] f32
        sums = out[:, :C]             # [K, C]
        counts = out[:, C]            # [K]
        protos.append(sums / (counts + EPS)[:, None])   # [K, C]
    return np.mean(np.stack(protos), axis=0).astype(np.float32)


# revision 30
# speedup vs baseline: 1.1599x; 1.1347x over previous
"""Trainium2 Bass kernel for nn_Correct_PrototypeManager (segment_reduce).

Reference computation:
    pred_lbl = argmax(preds, axis=1)                      # [B, H, W]
    feats_up = bilinear_resize(feats, H, W)               # [B, C, H, W]
    joint[b,k,h,w] = (masks==k) & (pred_lbl==k)
    counts[b,k] = sum_hw joint ; sums[b,k,c] = sum_hw feats_up * joint
    proto = mean_b( sums / (counts + eps) )               # [K, C]

Key algebraic transform: bilinear upsample is linear, so instead of
upsampling feats we DOWNSAMPLE the one-hot joint map with the adjoint
of the upsample (rows of U sum to 1, so counts are preserved exactly;
U's weights are quarters, so all downsample arithmetic is exact in
bf16 + f32 psum accumulation).

v2 layout strategy (all host prep is layout/cast only):
  - preds shipped pre-transposed to [hf, k, wf] so the DMA is fully
    contiguous and the argmax runs on natural views.
  - feats shipped bf16, pre-transposed to pixel-major [128, 32, 260]
    chunks with a ones-column at col 256 so the final contraction
    produces sums AND counts in one psum, with no on-device
    transposes of feats at all.
  - front-end (argmax eq / mask one-hot / joint) is split by class
    range across the DVE and GpSimd engines, which run in parallel.
  - final contraction is out[k, c]-oriented: 32 matmuls of 260 moving
    cols in 4 parallel psum chains.

Sharding: data-parallel over batch B=8, one image per NeuronCore; the
[K, C+1] per-image partial (sums | counts) is gathered on host,
divided and batch-meaned there (tiny).
"""

import numpy as np

B = 8
C = 256
K = 21
HC = WC = 64
HF = WF = 256
EPS = 1e-6
N_CORES = 8
PIX = HC * WC  # 4096
KW = K * WF    # 5376
K2 = K + 1     # 22 (pad class dim to even)
HK = HC * K2   # 1408
FTW = C + 4    # 260: feats cols + ones col (256) + pad
KSPL = 13      # eq/mul classes 0:13 on DVE, 13:21 on gpsimd

_PROGRAM_CACHE: dict = {}


def _upsample_matrix(n_in: int, n_out: int) -> np.ndarray:
    """U [n_out, n_in] with resize(x, 'bilinear', half-pixel) == U @ x."""
    U = np.zeros((n_out, n_in), dtype=np.float64)
    scale = n_in / n_out
    for i in range(n_out):
        src = (i + 0.5) * scale - 0.5
        f = int(np.floor(src))
        w = src - f
        lo = min(max(f, 0), n_in - 1)
        hi = min(max(f + 1, 0), n_in - 1)
        U[i, lo] += 1.0 - w
        U[i, hi] += w
    return U.astype(np.float32)


def _build_program(stage: int = 99):
    import concourse.bass as bass
    import concourse.bacc as bacc
    import concourse.tile as tile
    from concourse import mybir
    from contextlib import ExitStack

    f32 = mybir.dt.float32
    bf16 = mybir.dt.bfloat16

    nc = bacc.Bacc("TRN2", target_bir_lowering=False, debug=False,
                   num_devices=N_CORES)

    preds_d = nc.dram_tensor("preds", [2, 128, KW], f32,
                             kind="ExternalInput")
    mio_d = nc.dram_tensor("mio", [2, 128, WF + K], bf16,
                           kind="ExternalInput")
    ft_d = nc.dram_tensor("ft", [128, 32 * FTW], bf16, kind="ExternalInput")
    u_d = nc.dram_tensor("u", [2, 128, HC], bf16, kind="ExternalInput")
    out_d = nc.dram_tensor("out", [K, C + 1], f32, kind="ExternalOutput")
    if stage < 99:
        dbg_d = nc.dram_tensor("dbg", [128, KW], f32, kind="ExternalOutput")

    with tile.TileContext(nc) as tc, ExitStack() as ctx:
        const_pool = ctx.enter_context(tc.tile_pool(name="const", bufs=1))
        joint_pool = ctx.enter_context(tc.tile_pool(name="joint", bufs=1))
        res_pool = ctx.enter_context(tc.tile_pool(name="res", bufs=1))

        # --- constants / inputs ---
        # sync's HWDGE queue carries the big transfers in priority order
        # (preds_h0 first); gpsimd's SWDGE queue carries the small tiles
        # in parallel so they don't delay preds_h0.
        preds_t = []
        with tc.tile_pool(name="preds", bufs=1) as pr_pool:
            for h in range(2):
                pt = pr_pool.tile([128, KW], f32, tag=f"preds{h}")
                nc.sync.dma_start(pt[:], preds_d.ap()[h, :, :])
                preds_t.append(pt)
            ft_big = const_pool.tile([128, 32 * FTW], bf16, tag="ftbig")
            nc.sync.dma_start(ft_big[:], ft_d.ap()[:, :])
            mio_t = []
            for h in range(2):
                mt = const_pool.tile([128, WF + K], bf16, tag=f"mio{h}")
                nc.gpsimd.dma_start(mt[:], mio_d.ap()[h, :, :])
                mio_t.append(mt)
            u_t = []
            for h in range(2):
                ut = const_pool.tile([128, HC], bf16, tag=f"u{h}")
                nc.gpsimd.dma_start(ut[:], u_d.ap()[h, :, :])
                u_t.append(ut)

            # iota replicated [128, (k w)] bf16 via gpsimd memsets (idle
            # engine, runs during DMA); mask replicated per half via ACT
            # broadcast copies (also in the DMA shadow). With both
            # operands materialized the one-hot is a plain bf16 TT at
            # 2 elem/lane/cycle instead of broadcast ops at 1.
            iota_rep = const_pool.tile([128, KW], bf16, tag="iotarep")
            i3 = iota_rep[:].rearrange("p (k w) -> p k w", k=K)
            for k in range(K):
                nc.gpsimd.memset(i3[:, k, :], float(k))

            # --- front-end: joint one-hot map (DVE), ACT preps mask_rep ---
            joint_t = []
            with tc.tile_pool(name="fe", bufs=1) as fe_pool:
                for h in range(2):
                    p3 = preds_t[h][:].rearrange("p (k w) -> p k w", k=K)
                    mask_t = mio_t[h]

                    # materialized mask replica (ACT, in the DMA shadow)
                    mrep = fe_pool.tile([128, KW], bf16, tag="mrep")
                    nc.scalar.copy(
                        mrep[:].rearrange("p (k w) -> p k w", k=K),
                        mask_t[:, :WF].unsqueeze(1).to_broadcast(
                            [128, K, WF]))
                    oh_t = fe_pool.tile([128, KW], bf16, tag="oh")
                    nc.vector.tensor_tensor(
                        oh_t[:], mrep[:], iota_rep[:],
                        op=mybir.AluOpType.is_equal)

                    # TT-max tree over classes (contiguous views)
                    mx = fe_pool.tile([128, 11 * WF], f32, tag="mx")
                    m3 = mx[:].rearrange("p (k w) -> p k w", k=11)
                    nc.vector.tensor_tensor(
                        m3[:, 0:10, :], p3[:, 0:10, :], p3[:, 11:21, :],
                        op=mybir.AluOpType.max)
                    nc.vector.tensor_copy(m3[:, 10, :], p3[:, 10, :])
                    nc.vector.tensor_tensor(
                        m3[:, 0:5, :], m3[:, 0:5, :], m3[:, 6:11, :],
                        op=mybir.AluOpType.max)
                    nc.vector.tensor_tensor(
                        m3[:, 0:3, :], m3[:, 0:3, :], m3[:, 3:6, :],
                        op=mybir.AluOpType.max)
                    nc.vector.tensor_tensor(
                        m3[:, 0, :], m3[:, 0, :], m3[:, 1, :],
                        op=mybir.AluOpType.max)
                    nc.vector.tensor_tensor(
                        m3[:, 0, :], m3[:, 0, :], m3[:, 2, :],
                        op=mybir.AluOpType.max)
                    maxv = m3[:, 0, :]

                    eq_t = fe_pool.tile([128, KW], bf16, tag="eq")
                    jt = joint_pool.tile([128, KW], bf16, tag=f"joint{h}")
                    e3 = eq_t[:].rearrange("p (k w) -> p k w", k=K)
                    nc.vector.tensor_tensor(
                        e3[:, :, :], p3[:, :, :],
                        maxv.unsqueeze(1).to_broadcast([128, K, WF]),
                        op=mybir.AluOpType.is_equal)
                    nc.vector.tensor_tensor(
                        jt[:], eq_t[:], oh_t[:], op=mybir.AluOpType.mult)
                    joint_t.append(jt)

            if stage <= 1:  # debug: dump joint (cast bf16->f32 via gpsimd)
                nc.gpsimd.dma_start(dbg_d.ap()[:, :], joint_t[0][:, :])

        # ----- stage 1 (transposed): AT[(k,wf) chunk, hc] = joint^T @ Uh --
        # 42 chunks of 128 (k,wf)-columns per half; both halves accumulate
        # into the same psum (start on h0, stop on h1), so h0's matmuls run
        # during h1's front-end and no separate transpose pass is needed.
        # Psum bank b holds 8 chunks side by side -> one strided copy per
        # bank into at_big's (w, hc, k) layout.
        with tc.tile_pool(name="stage", bufs=1) as st_pool:
            at_big = st_pool.tile([128, 2 * HK], bf16, tag="at")
            if stage >= 2:
                # zero the k=21 pad column so stage 2 produces clean zeros
                nc.gpsimd.memset(
                    at_big[:].rearrange(
                        "p (w h k) -> p w h k", w=2, h=HC)[:, :, :, K], 0.0)
            NCH = KW // 128  # 42
            # NOTE: psum accumulation groups (start on h0 / stop on h1)
            # interleaved with other matmuls corrupt results on HW, so each
            # half runs complete groups; h0 is parked in SBUF (ACT copies,
            # hidden under h1's front-end) and DVE merges h0+h1 at the end.
            at_sb = st_pool.tile([128, 6 * 512], bf16, tag="atsb")
            atv = at_big[:].rearrange("p (w h k) -> p k w h", w=2, h=HC)
            with tc.tile_pool(name="atps", bufs=1, space="PSUM") as at_pool:
                for h in range(2):
                    at_ps = []
                    if stage >= 2:
                        for b in range(6):
                            pt = at_pool.tile([128, 512], f32, tag=f"at{b}")
                            at_ps.append(pt)
                    for q in range(NCH if stage >= 2 else 0):
                        nc.tensor.matmul(
                            at_ps[q // 8][:, 64 * (q % 8):64 * (q % 8) + 64],
                            joint_t[h][:, 128 * q:128 * q + 128],
                            u_t[h][:, :],
                            start=True, stop=True)
                    for b in range(6 if stage >= 2 else 0):
                        nk = 4 if b < 5 else 1  # bank 5: class 20 only
                        nc_ = nk * 128
                        if h == 0:
                            nc.scalar.copy(at_sb[:, 512 * b:512 * b + nc_],
                                           at_ps[b][:, 0:nc_])
                        else:
                            src0 = at_sb[:, 512 * b:512 * b + nc_].rearrange(
                                "p (k w h) -> p k w h", k=nk, w=2)
                            src1 = at_ps[b][:, 0:nc_].rearrange(
                                "p (k w h) -> p k w h", k=nk, w=2)
                            nc.vector.tensor_tensor(
                                atv[:, 4 * b:4 * b + nk, :, :], src0, src1,
                                op=mybir.AluOpType.add)

            # ----- stage 2: contract wf.  B[wc, (hc, k)] = Uw^T @ AT -----
            # B lives twice (partitions 0-63 and 64-127) so the final
            # matmuls can match the base partition of the FT slice.
            # 4 chunks of 352 cols (16 hc) aligned with the 4 final psum
            # chains; the partition-shift dup runs per chunk on gpsimd so
            # chain c can start right after dup c.
            ps_pool = ctx.enter_context(
                tc.tile_pool(name="ps", bufs=2, space="PSUM"))
            psf_pool = ctx.enter_context(
                tc.tile_pool(name="psf", bufs=1, space="PSUM"))
            b_t = st_pool.tile([128, HK], bf16, tag="b")
            CW = HK // 4  # 352
            for fc in range(0, HK if stage >= 3 else 0, CW):
                ps = ps_pool.tile([64, 512], f32, tag="ps")
                nc.tensor.matmul(ps[:, :CW], u_t[0][:, :],
                                 at_big[:, fc:fc + CW],
                                 start=True, stop=False)
                nc.tensor.matmul(ps[:, :CW], u_t[1][:, :],
                                 at_big[:, HK + fc:HK + fc + CW],
                                 start=False, stop=True)
                nc.scalar.copy(b_t[0:64, fc:fc + CW], ps[:, :CW])
                # dup for this chunk: dst cols [fc-K2, fc+CW-K2) from
                # src cols [fc, fc+CW) shifted by one hc (K2 cols)
                d0 = max(fc - K2, 0)
                nc.gpsimd.dma_start(b_t[64:128, d0:fc + CW - K2],
                                    b_t[0:64, d0 + K2:fc + CW])
            if stage == 3:
                nc.gpsimd.dma_start(dbg_d.ap()[:, 0:HK], b_t[:, :])

            # ----- final: out[k, c|cnt] = sum_pix B[pix,k] (x) FT[pix,:] --
            # 4 chains emitted round-robin; chain c consumes b_t chunk c.
            ftv = ft_big[:].rearrange("p (x n) -> p x n", x=32)
            psf_t = []
            if stage >= 4:
                for c in range(4):
                    psf = psf_pool.tile([K2, FTW], f32, tag=f"fin{c}")
                    psf_t.append(psf)
                for i in range(8):
                    for c in range(4):
                        ch = c * 8 + i
                        nc.tensor.matmul(
                            psf_t[c][:, :],
                            b_t[:, 2 * ch * K2: 2 * ch * K2 + K2],
                            ftv[:, ch, :],
                            start=(i == 0), stop=(i == 7))

            if stage >= 4:
                # TT may read only one PSUM operand: copy then chain adds
                s01 = res_pool.tile([K2, FTW], f32, tag="s01")
                nc.vector.tensor_copy(s01[:], psf_t[0][:])
                for i in (1, 2, 3):
                    nc.vector.tensor_tensor(s01[:], s01[:], psf_t[i][:],
                                            op=mybir.AluOpType.add)
                nc.sync.dma_start(out_d.ap()[:, :], s01[0:K, 0:C + 1])

    nc.compile()
    return nc


def _get_program(stage: int = 99):
    key = f"nc{stage}"
    if key not in _PROGRAM_CACHE:
        _PROGRAM_CACHE[key] = _build_program(stage)
    return _PROGRAM_CACHE[key]


def _host_inputs(feats, preds, masks):
    import ml_dtypes

    U = _upsample_matrix(HC, HF)  # [256, 64], exact in bf16
    u_pack = np.ascontiguousarray(
        U.reshape(2, 128, HC)).astype(ml_dtypes.bfloat16)

    feats = np.asarray(feats, dtype=np.float32)
    preds = np.asarray(preds, dtype=np.float32)
    masks_f = np.asarray(masks).astype(np.float32)
    iota_row = np.arange(K, dtype=np.float32)
    # [B, 2, 128, WF+K]: mask halves with the iota row appended
    mio = np.empty((B, 2, 128, WF + K), dtype=np.float32)
    mio[..., :WF] = masks_f.reshape(B, 2, 128, WF)
    mio[..., WF:] = iota_row
    mio_bf = mio.astype(ml_dtypes.bfloat16)

    # preds pre-transposed to [hf, k, wf], split into halves
    preds_hf = np.ascontiguousarray(
        preds.transpose(0, 2, 1, 3)).reshape(B, 2, 128, KW)

    # feats: [C, PIX] -> pixel-major chunks [128, 32, FTW] bf16 + ones col
    ft_host = np.zeros((B, 128, 32, FTW), dtype=np.float32)
    ftt = feats.reshape(B, C, PIX).transpose(0, 2, 1)   # [B, PIX, C]
    ft_host[..., :C] = ftt.reshape(B, 32, 128, C).transpose(0, 2, 1, 3)
    ft_host[..., C] = 1.0
    ft_bf = ft_host.reshape(B, 128, 32 * FTW).astype(ml_dtypes.bfloat16)

    in_maps = []
    for b in range(B):
        in_maps.append({
            "preds": np.ascontiguousarray(preds_hf[b]),
            "mio": np.ascontiguousarray(mio_bf[b]),
            "ft": np.ascontiguousarray(ft_bf[b]),
            "u": u_pack,
        })
    return in_maps


def kernel(feats, preds, masks, _results_hook=None, _stage=99):
    from concourse.bass_utils import run_bass_kernel_spmd

    nc = _get_program(_stage)
    in_maps = _host_inputs(feats, preds, masks)
    res = run_bass_kernel_spmd(nc, in_maps, list(range(N_CORES)))
    if _results_hook is not None:
        _results_hook(res)

    protos = []
    for b in range(B):
        out = res.results[b]["out"]   # [K, C+1] f32
        sums = out[:, :C]             # [K, C]
        counts = out[:, C]            # [K]
        protos.append(sums / (counts + EPS)[:, None])   # [K, C]
    return np.mean(np.stack(protos), axis=0).astype(np.float32)
